# revision 65
# baseline (speedup 1.0000x reference)
"""ALiBi transformer layer on 8 TRN2 NeuronCores.

Sharding: 2 batch groups x 4 cores. Core c (b=c//4, r=c%4) handles 4 heads
(Megatron column split) for batch b, then the 512-token slice r of out_proj
/ LN2 / FFN.

Key design points (v2):
  - All matmuls run in bf16 (same PE column rate as fp32r, but FWL weight
    loads, half the DMA, half the SBUF). PSUM accumulation stays fp32.
  - LN1 and LN2 are folded into the following matmuls: stats are computed
    with an all-ones [128,128] stationary (so mean/rstd land broadcast on
    all 128 partitions with no gpsimd broadcast), the -mu*colsum(W) rank-1
    term is added via a K=1 matmul row, and the *rstd scaling is applied to
    the matmul outputs.  Matmuls therefore never wait on a normalize pass.
  - Attention is software-pipelined per key-block: scores(kb+1) is issued
    before PV(kb), so the PE never idles while the scalar engine computes
    exp (keeps the HAM clock gate at 8/8).  Softmax uses no row-max: the
    alibi bias -slope*j keeps exp bounded, denominators come from an
    appended ones column in V.
  - Head outputs are redistributed with per-head-pair AllToAll (4x less
    wire than AllGather) in a layout where each received [128,512] block is
    exactly one out_proj contraction chunk - no indirect gather.
  - W1/W2 (bf16) are prefetched during attention / FFN1 respectively.
"""
import numpy as np
import ml_dtypes

import concourse.bass as bass
import concourse.tile as tile
import concourse.mybir as mybir
from concourse import bacc
from concourse.bass_utils import run_bass_kernel_spmd

B, S, D, H, DH, FF = 2, 2048, 1024, 16, 64, 4096
NCORES, GROUP = 8, 4
HPC = H // GROUP            # heads per core = 4
SL = S // GROUP             # token slice per core = 512
EPS = 1e-5
F32 = mybir.dt.float32
BF16 = mybir.dt.bfloat16
AF = mybir.ActivationFunctionType
ALU = mybir.AluOpType
ts = bass.ts


def build_nc(use_cqk: bool, use_cv: bool, use_c1: bool, use_b2: bool,
             dbg: bool = False):
    nc = bacc.Bacc("TRN2", target_bir_lowering=False, debug=False)

    srcTb = nc.dram_tensor("srcTb", (D, S), BF16, kind="ExternalInput")
    srcTs = nc.dram_tensor("srcTs", (D, SL), F32, kind="ExternalInput")
    wqkT = nc.dram_tensor("wqkT", (D, 2 * HPC * DH), BF16, kind="ExternalInput")
    wvT = nc.dram_tensor("wvT", (D, HPC * DH), BF16, kind="ExternalInput")
    woutT = nc.dram_tensor("woutT", (D, D), BF16, kind="ExternalInput")
    w1T = nc.dram_tensor("w1T", (D, FF), BF16, kind="ExternalInput")
    w2T = nc.dram_tensor("w2T", (FF, D), BF16, kind="ExternalInput")
    alibi = nc.dram_tensor("alibi", (128, HPC * 16), F32, kind="ExternalInput")
    tri = nc.dram_tensor("tri", (128, 128), BF16, kind="ExternalInput")
    csqk = nc.dram_tensor("csqk", (1, 2 * HPC * DH), BF16, kind="ExternalInput")
    csv = nc.dram_tensor("csv", (1, HPC * DH), F32, kind="ExternalInput")
    cs1 = nc.dram_tensor("cs1", (1, FF), BF16, kind="ExternalInput")
    cqk = nc.dram_tensor("cqk", (128, 4), F32, kind="ExternalInput")
    cv = nc.dram_tensor("cv", (1, HPC * DH), F32, kind="ExternalInput")
    c1 = nc.dram_tensor("c1", (128, 32), F32, kind="ExternalInput")
    b2c = nc.dram_tensor("b2c", (128, 8), F32, kind="ExternalInput")
    gidx = nc.dram_tensor("gidx", (128, 8), mybir.dt.int32, kind="ExternalInput")
    gidxd = nc.dram_tensor("gidxd", (8, 2), mybir.dt.int32, kind="ExternalInput")
    selb = nc.dram_tensor("selb", (8, 8 * 128), BF16, kind="ExternalInput")
    outT = nc.dram_tensor("outT", (D, SL), F32, kind="ExternalOutput")
    if dbg:
        d_qkT = nc.dram_tensor("d_qkT", (128, 4, S), BF16, kind="ExternalOutput")
        d_v = nc.dram_tensor("d_v", (128, 16, HPC, DH + 1), BF16,
                             kind="ExternalOutput")
        d_oh = nc.dram_tensor("d_oh", (DH + 1, HPC, S), BF16,
                              kind="ExternalOutput")
        d_ot = nc.dram_tensor("d_ot", (128, 8, SL), BF16, kind="ExternalOutput")
        d_src2 = nc.dram_tensor("d_src2", (128, 8, SL), F32,
                                kind="ExternalOutput")

    RG8 = [[0, 1, 2, 3, 4, 5, 6, 7]]

    with tile.TileContext(nc) as tc:
        # ---- persistent small constants ----
        consts = tc.alloc_tile_pool(name="consts", bufs=1)
        alibi_sb = consts.tile([128, HPC * 16], F32)
        nc.sync.dma_start(alibi_sb, alibi.ap())
        tri_sb = consts.tile([128, 128], BF16)
        nc.sync.dma_start(tri_sb, tri.ap())
        csqk_sb = consts.tile([1, 2 * HPC * DH], BF16)
        nc.sync.dma_start(csqk_sb, csqk.ap())
        ones_sb = consts.tile([128, 128], BF16)
        nc.vector.memset(ones_sb, 1.0)
        eps_sb = consts.tile([1, 1], F32)
        nc.vector.memset(eps_sb, EPS)
        epsB_sb = consts.tile([128, 1], F32)
        nc.vector.memset(epsB_sb, EPS)
        csvB = consts.tile([128, HPC * DH], F32)
        csv_row = consts.tile([1, HPC * DH], F32)
        nc.sync.dma_start(csv_row, csv.ap())
        nc.gpsimd.partition_broadcast(csvB, csv_row)
        if use_cqk:
            cqk_sb = consts.tile([128, 4], F32)
            nc.sync.dma_start(cqk_sb, cqk.ap())
        if use_cv:
            cv_row = consts.tile([1, HPC * DH], F32)
            nc.sync.dma_start(cv_row, cv.ap())
            cvB = consts.tile([128, HPC * DH], F32)
            nc.gpsimd.partition_broadcast(cvB, cv_row)
        if use_c1:
            c1_sb = consts.tile([128, 32], F32)
            nc.sync.dma_start(c1_sb, c1.ap())
        if use_b2:
            b2_sb = consts.tile([128, 8], F32)
            nc.sync.dma_start(b2_sb, b2c.ap())
        gidx_sb = consts.tile([128, 8], mybir.dt.int32)
        nc.sync.dma_start(gidx_sb, gidx.ap())
        gidxd_sb = consts.tile([8, 2], mybir.dt.int32)
        nc.sync.dma_start(gidxd_sb, gidxd.ap())
        # tiny warm-up AllToAll: the FIRST collective pays a large one-time
        # entry latency (~50-80us); absorb it here, overlapped with P1/P2
        warm_dram = tc.alloc_tile_pool(name="warmd", bufs=1, space="DRAM")
        warm_in = warm_dram.tile([8, 128], BF16, name="warm_in")
        warm_out = warm_dram.tile([8, 128], BF16, name="warm_out")
        warm_sb = consts.tile([8, 128], BF16)
        nc.vector.memset(warm_sb, 0.0)
        nc.sync.dma_start(warm_in, warm_sb)
        nc.gpsimd.collective_compute(
            "AllToAll", ALU.bypass, replica_groups=[[0, 1, 2, 3, 4, 5, 6, 7]],
            ins=[warm_in.opt()], outs=[warm_out.opt()])
        selb_sb = consts.tile([8, 8 * 128], BF16)
        nc.sync.dma_start(selb_sb, selb.ap())

        # qkT / v live from QKV until end of attention
        mids = tc.alloc_tile_pool(name="mids", bufs=1)
        qkT_sb = mids.tile([128, 4, S], BF16)        # [dh(2 heads), grp, t]
        v_sb = mids.tile([128, 16, HPC, DH + 1], BF16)  # [tok_p, tok_tile, h, dh|1]
        nc.gpsimd.memset(v_sb[:, :, :, DH:DH + 1], 1.0)

        # =============== P1: load + LN1 stats ===============
        p1big = tc.alloc_tile_pool(name="p1big", bufs=1)
        x_sb = p1big.tile([128, 8, S], BF16)
        srcT_v = srcTb.ap().rearrange("(g p) t -> p g t", p=128)
        for c in range(8):
            nc.sync.dma_start(x_sb[:, c, :], srcT_v[:, c, :])

        ln1 = tc.alloc_tile_pool(name="ln1", bufs=1)
        muB = ln1.tile([128, S], F32)       # mean, broadcast on all partitions
        rB = ln1.tile([128, S], F32)        # rstd, broadcast on all partitions
        nmu_bf = ln1.tile([1, S], BF16)     # -mu row (fold-matmul rhs)
        rc_cols = ln1.tile([128, 2, 16], F32)   # rstd / -mu as columns per tt
        rc_dram = tc.alloc_tile_pool(name="rcd", bufs=1, space="DRAM")
        rc_stage = rc_dram.tile([2, S], F32)
        # QKV weights: DMA queued right behind the src chunks
        wqk_sb = ln1.tile([128, 8, 2 * HPC * DH], BF16)
        nc.sync.dma_start(wqk_sb, wqkT.ap().rearrange("(g p) f -> p g f", p=128))
        wv_sb = ln1.tile([128, 8, HPC * DH], BF16)
        nc.sync.dma_start(wv_sb, wvT.ap().rearrange("(g p) f -> p g f", p=128))

        with (
            tc.tile_pool(name="p1sq", bufs=2) as p1sq,
            tc.tile_pool(name="p1ps", bufs=1, space="PSUM") as p1ps,
        ):
            st_x = p1ps.tile([128, 4, 512], F32)
            st_x2 = p1ps.tile([128, 4, 512], F32)
            for c in range(8):
                sq = p1sq.tile([128, S], BF16, tag="sq")
                xs = x_sb[:, c, :]
                if c % 2 == 0:
                    nc.vector.tensor_mul(sq, xs, xs)
                else:
                    nc.scalar.activation(sq, xs, AF.Square)
                for qg in range(4):
                    nc.tensor.matmul(st_x[:, qg, :], ones_sb,
                                     x_sb[:, c, ts(qg, 512)],
                                     start=(c == 0), stop=(c == 7))
                    nc.tensor.matmul(st_x2[:, qg, :], ones_sb,
                                     sq[:, ts(qg, 512)],
                                     start=(c == 0), stop=(c == 7))
            stx_flat = st_x.rearrange("p g t -> p (g t)")
            stx2_flat = st_x2.rearrange("p g t -> p (g t)")
            # muB released first so QKV matmuls can start sooner
            nc.vector.tensor_scalar_mul(muB, stx_flat, 1.0 / D)
            var = rB  # reuse storage
            nc.vector.scalar_tensor_tensor(var, muB, -1.0, muB,
                                           op0=ALU.mult, op1=ALU.mult)
            nc.vector.scalar_tensor_tensor(var, stx2_flat, 1.0 / D, var,
                                           op0=ALU.mult, op1=ALU.add)
        # rstd = exp(-0.5*ln(var+eps)); all on 128 partitions already
        nc.scalar.activation(rB, rB, AF.Ln, bias=epsB_sb, scale=1.0)
        nc.scalar.activation(rB, rB, AF.Exp, bias=0.0, scale=-0.5)
        nc.vector.tensor_scalar_mul(nmu_bf, muB[0:1, :], -1.0)
        # roundtrip rstd/-mu rows into token-on-partition columns (v post-op)
        nc.sync.dma_start(rc_stage[0:1, :], rB[0:1, :])
        nc.sync.dma_start(rc_stage[1:2, :], muB[0:1, :])
        nc.sync.dma_start(rc_cols,
                          rc_stage.rearrange("j (tt p) -> p j tt", p=128))

        # =============== P2: QKV (LN1 folded in) ===============
        # all post-scales live on the vector engine: gpsimd is ~8x slower on
        # these APs, and the scalar engine must keep its EXP table loaded
        with (
            tc.tile_pool(name="p2ps", bufs=4, space="PSUM") as p2ps,
            tc.tile_pool(name="p2psv", bufs=2, space="PSUM") as p2psv,
        ):
            for blk in range(4):
                for qg in range(4):
                    qp = p2ps.tile([128, 512], F32, tag="qk")
                    for c in range(8):
                        nc.tensor.matmul(qp, wqk_sb[:, c, ts(blk, 128)],
                                         x_sb[:, c, ts(qg, 512)],
                                         start=(c == 0), stop=False)
                    # rank-1 fold: += (-mu) x colsum(W_blk)
                    nc.tensor.matmul(qp, csqk_sb[0:1, ts(blk, 128)],
                                     nmu_bf[0:1, ts(qg, 512)],
                                     start=False, stop=True)
                    dst = qkT_sb[:, blk, ts(qg, 512)]
                    nc.vector.tensor_mul(dst, qp, rB[:, ts(qg, 512)])
                    if use_cqk:
                        nc.vector.tensor_scalar_add(dst, dst,
                                                    cqk_sb[:, blk:blk + 1])
            for tt in range(16):
                vp = p2psv.tile([128, HPC * DH], F32, tag="v")
                for c in range(8):
                    nc.tensor.matmul(vp, x_sb[:, c, ts(tt, 128)],
                                     wv_sb[:, c, :],
                                     start=(c == 0), stop=(c == 7))
                # v = (psum + (-mu_t)*colsum_v) * rstd_t (+ cv)
                vdst = v_sb[:, tt, :, 0:DH]
                csvB3 = csvB.rearrange("p (h d) -> p h d", h=HPC)
                vp3 = vp.rearrange("p (h d) -> p h d", h=HPC)
                nc.vector.scalar_tensor_tensor(
                    vdst, csvB3, rc_cols[:, 1, tt:tt + 1], vp3,
                    op0=ALU.mult, op1=ALU.add)
                if use_cv:
                    nc.vector.scalar_tensor_tensor(
                        vdst, vdst, rc_cols[:, 0, tt:tt + 1],
                        cvB.rearrange("p (h d) -> p h d", h=HPC),
                        op0=ALU.mult, op1=ALU.add)
                else:
                    nc.vector.tensor_scalar_mul(vdst, vdst,
                                                rc_cols[:, 0, tt:tt + 1])
        if dbg:
            nc.sync.dma_start(d_qkT.ap(), qkT_sb)
            nc.sync.dma_start(d_v.ap(), v_sb)
        rc_dram.release()
        ln1.release()
        p1big.release()

        # W1 prefetch (bf16, 8MB) - trickles in during attention
        pw1 = tc.alloc_tile_pool(name="pw1", bufs=1, side="right")
        w1_sb = pw1.tile([128, 8, FF], BF16)
        for c in range(8):
            nc.sync.dma_start(w1_sb[:, c, :],
                              w1T.ap().rearrange("(g p) f -> p g f", p=128)[:, c, :])
        # wout + residual-slice prefetch
        p4w = tc.alloc_tile_pool(name="p4w", bufs=1, side="right")
        wout_sb = p4w.tile([128, 8, D], BF16)
        nc.sync.dma_start(wout_sb, woutT.ap().rearrange("(g p) f -> p g f", p=128))
        srcTs_sb = p4w.tile([128, 8, SL], F32)
        nc.sync.dma_start(srcTs_sb, srcTs.ap().rearrange("(g p) t -> p g t", p=128))

        # =============== P3: attention ===============
        # Head pairs share a column group with head A on partitions 0:64 and
        # head B on 64:128, so the pair's score matmuls are row-tiled and run
        # CONCURRENTLY in the PE array (2x effective score throughput).
        # Queries are processed in two passes (0:1024 then 1024:2048) so each
        # head only ever holds 2 live PV banks -> 4 PV + 4 score banks = 8.
        # Slot 3 hosts global heads 0-3 (steepest alibi slopes): key blocks
        # kb>3 contribute exp(<-40) of the kept mass and are skipped.
        # Outputs are sent UNNORMALIZED (65 rows: o and the exp-sum row);
        # normalization happens after the AllToAll in P4.
        dram = tc.alloc_tile_pool(name="dram", bufs=1, space="DRAM")
        a2a_ins = [dram.tile([2, GROUP, 2, DH + 1, SL], BF16, name=f"a2i{i}")
                   for i in range(2)]
        a2a_outs = [dram.tile([2 * GROUP, 2, DH + 1, SL], BF16, name=f"a2o{i}")
                    for i in range(2)]
        # P4-ingest tiles are allocated up front: the gathers are emitted
        # inside the pair loop, right behind each pair's collective
        p4in = tc.alloc_tile_pool(name="p4in", bufs=1, side="right")
        ot_sb = p4in.tile([128, 8, SL], BF16)
        denoms = [p4in.tile([8, SL], BF16, name=f"den{i}") for i in range(2)]
        recipfs = [p4in.tile([8, SL], F32, name=f"rcf{i}") for i in range(2)]
        recips = [p4in.tile([8, SL], BF16, name=f"rcp{i}") for i in range(2)]
        a2a_flats = [x.rearrange("s hh p t -> (s hh p) t") for x in a2a_outs]
        poh = tc.alloc_tile_pool(name="poh", bufs=1)
        # rows 0-63 oT, row 64 exp-sums (unnormalized)
        oh_sb = poh.tile([DH + 1, HPC, S], BF16)
        BCUT = 3  # slot-3 band: keep kb <= BCUT only

        with (
            tc.tile_pool(name="p3e", bufs=3) as p3e,
            tc.tile_pool(name="p3s", bufs=2, space="PSUM") as p3s,
            tc.tile_pool(name="p3pv", bufs=1, space="PSUM") as p3pv,
        ):
            for pair in range(2):
                qgrp, kgrp = pair, 2 + pair
                # head B of pair 1 is the banded slot-3 head
                cut = {0: 15, 1: BCUT}[pair]
                for pss in range(2):
                    if pair == 1 and pss == 1:
                        # pair-0's denominator reciprocal: den0 has landed by
                        # now and the vector FIFO has a bubble here, taking
                        # it off the post-attention critical chain
                        nc.vector.reciprocal(recipfs[0], denoms[0])
                        nc.vector.tensor_copy(recips[0], recipfs[0])
                    qlo = 1024 * pss
                    kbmax = 8 + 8 * pss
                    pvs = {}
                    for hh in (0, 1):
                        for j in (0, 1):
                            pvs[hh, j] = p3pv.tile(
                                [DH + 1, 512], F32, tag=f"pv{hh}{j}",
                                name=f"pv{pair}_{pss}_{hh}_{j}")
                    ets = [[None] * kbmax for _ in range(2)]
                    for kb in range(kbmax + 2):
                        if kb < kbmax:
                            qstart = max(qlo, 128 * kb)
                            W = qlo + 1024 - qstart
                            heads = (0, 1) if (kb <= cut) else (0,)
                            for hh in heads:
                                base = 64 * hh
                                et = p3e.tile([128, 1024], BF16,
                                              tag=f"et{hh}")
                                ets[hh][kb] = et
                                kT = qkT_sb[base:base + 64, kgrp, ts(kb, 128)]
                                for sc in range((W + 511) // 512):
                                    w = min(512, W - 512 * sc)
                                    off = qstart + 512 * sc
                                    sp = p3s.tile([128, 512], F32,
                                                  tag=f"sc{hh}")
                                    nc.tensor.matmul(
                                        sp[:, :w], kT,
                                        qkT_sb[base:base + 64, qgrp,
                                               off:off + w],
                                        start=True, stop=True)
                                    h4 = 2 * pair + hh
                                    nc.scalar.activation(
                                        et[:, 512 * sc:512 * sc + w],
                                        sp[:, :w], AF.Exp,
                                        bias=alibi_sb[:, h4 * 16 + kb:
                                                      h4 * 16 + kb + 1],
                                        scale=0.125)
                                if qstart == 128 * kb:
                                    # keep gpsimd EMPTY in P3: the P4 ingest
                                    # gathers share its FIFO and must start
                                    # the moment each AllToAll lands
                                    nc.vector.tensor_mul(et[:, 0:128],
                                                         et[:, 0:128], tri_sb)
                        if kb >= 2 and kb - 2 < kbmax:
                            pkb = kb - 2
                            qstart2 = max(qlo, 128 * pkb)
                            for hh in (0, 1):
                                if ets[hh][pkb] is None:
                                    continue
                                pet = ets[hh][pkb]
                                for j in (0, 1):
                                    qgbase = qlo + 512 * j
                                    if qgbase + 512 <= qstart2:
                                        continue
                                    ostart = max(0, qstart2 - qgbase)
                                    estart = max(0, qgbase - qstart2)
                                    n = 512 - ostart
                                    last = min(kbmax - 1,
                                               (qgbase + 512) // 128 - 1)
                                    if hh == 1:
                                        last = min(last, cut)
                                    nc.tensor.matmul(
                                        pvs[hh, j][:, ostart:512],
                                        v_sb[:, pkb, 2 * pair + hh, :],
                                        pet[:, estart:estart + n],
                                        start=(pkb == 0), stop=(pkb == last))
                    for hh in (0, 1):
                        for j in (0, 1):
                            nc.vector.tensor_copy(
                                oh_sb[0:DH + 1, 2 * pair + hh,
                                      qlo + 512 * j:qlo + 512 * j + 512],
                                pvs[hh, j])
                # fire the pair's AllToAll (unnormalized, 65 rows).
                # Same payload into both group-halves (SPMD: no core id;
                # peers outside the group discard).
                for hh in (0, 1):
                    src_v = oh_sb[0:DH + 1, 2 * pair + hh, :].rearrange(
                        "p (tb t) -> p tb t", tb=GROUP)
                    for gg in range(2):
                        nc.sync.dma_start(
                            a2a_ins[pair][gg, :, hh].rearrange(
                                "tb p t -> p tb t"),
                            src_v)
                nc.gpsimd.collective_compute(
                    "AllToAll", ALU.bypass,
                    replica_groups=RG8,
                    ins=[a2a_ins[pair].opt()],
                    outs=[a2a_outs[pair].opt()])
                # ingest gathers, queued directly behind this pair's
                # collective so they run the moment it lands
                for c in range(4 * pair, 4 * pair + 4):
                    nc.gpsimd.indirect_dma_start(
                        out=ot_sb[:, c, :], out_offset=None,
                        in_=a2a_flats[pair],
                        in_offset=bass.IndirectOffsetOnAxis(
                            ap=gidx_sb[:, c:c + 1], axis=0))
                nc.gpsimd.indirect_dma_start(
                    out=denoms[pair], out_offset=None,
                    in_=a2a_flats[pair],
                    in_offset=bass.IndirectOffsetOnAxis(
                        ap=gidxd_sb[0:8, pair:pair + 1], axis=0))

        poh.release()
        mids.release()

        # =============== P4: out_proj + residual + LN2 stats ===============
        p46 = tc.alloc_tile_pool(name="p46", bufs=1)
        src2T_sb = p46.tile([128, 8, SL], F32)
        src2b_sb = p46.tile([128, 8, SL], BF16)
        ln2 = tc.alloc_tile_pool(name="ln2", bufs=1)
        r2B = ln2.tile([128, SL], F32)
        mu2 = ln2.tile([128, SL], F32)
        with (
            tc.tile_pool(name="p4t", bufs=1) as p4t,
            tc.tile_pool(name="p4ps", bufs=2, space="PSUM") as p4ps,
            tc.tile_pool(name="p4ps2", bufs=1, space="PSUM") as p4ps2,
            tc.tile_pool(name="p4rb", bufs=2, space="PSUM") as p4rb,
            tc.tile_pool(name="p4sq", bufs=2) as p4sq,
        ):
            st2x = p4ps2.tile([128, 512], F32)
            st2x2 = p4ps2.tile([128, 512], F32)
            oa_sb = p4t.tile([128, 8, SL], BF16)
            # two chunk-passes: pass A contracts the pair-0 chunks (c 0..3)
            # for ALL 8 blocks into SBUF partials while the pair-1 AllToAll
            # is still in flight; pass B adds the pair-1 chunks
            for wave, cs in enumerate(([0, 1, 2, 3], [4, 5, 6, 7])):
                # normalize wave chunks once (before first use):
                # rb = broadcast of recip rows via K=8 matmul
                # (pair-0's reciprocal was computed back in P3)
                if wave == 1:
                    nc.vector.reciprocal(recipfs[1], denoms[1])
                    nc.vector.tensor_copy(recips[1], recipfs[1])
                for c in cs:
                    rbp = p4rb.tile([128, SL], F32, tag="rb")
                    nc.tensor.matmul(rbp, selb_sb[:, ts(c, 128)],
                                     recips[c // 4],
                                     start=True, stop=True)
                    nc.vector.tensor_mul(ot_sb[:, c, :],
                                         ot_sb[:, c, :], rbp)
                for blk in range(8):
                    op = p4ps.tile([128, SL], F32, tag="op", bufs=4,
                                   name=f"op{wave}_{blk}")
                    for c in cs:
                        nc.tensor.matmul(op, wout_sb[:, c, ts(blk, 128)],
                                         ot_sb[:, c, :],
                                         start=(c == cs[0]),
                                         stop=(c == cs[3]))
                    if wave == 0:
                        nc.vector.tensor_copy(oa_sb[:, blk, :], op)
                    else:
                        nc.vector.tensor_add(op, op, oa_sb[:, blk, :])
                        nc.vector.tensor_add(src2T_sb[:, blk, :], op,
                                             srcTs_sb[:, blk, :])
                        nc.gpsimd.tensor_copy(src2b_sb[:, blk, :],
                                              src2T_sb[:, blk, :])
                        sq2 = p4sq.tile([128, SL], BF16, tag="sq2")
                        nc.gpsimd.tensor_mul(sq2, src2b_sb[:, blk, :],
                                             src2b_sb[:, blk, :])
                        nc.tensor.matmul(st2x, ones_sb,
                                         src2b_sb[:, blk, :],
                                         start=(blk == 0), stop=(blk == 7))
                        nc.tensor.matmul(st2x2, ones_sb, sq2,
                                         start=(blk == 0), stop=(blk == 7))
            if dbg:
                nc.sync.dma_start(d_ot.ap(), ot_sb)
                nc.sync.dma_start(d_src2.ap(), src2T_sb)
            nc.vector.tensor_scalar_mul(mu2, st2x, 1.0 / D)
            var2 = p4sq.tile([128, SL], F32, tag="v2", bufs=1)
            nc.vector.scalar_tensor_tensor(var2, mu2, -1.0, mu2,
                                           op0=ALU.mult, op1=ALU.mult)
            nc.vector.scalar_tensor_tensor(var2, st2x2, 1.0 / D, var2,
                                           op0=ALU.mult, op1=ALU.add)
            nc.scalar.activation(r2B, var2, AF.Ln, bias=epsB_sb, scale=1.0)
            nc.scalar.activation(r2B, r2B, AF.Exp, bias=0.0, scale=-0.5)
            mu2b = p4sq.tile([128, SL], BF16, tag="m2b", bufs=1)
            r2Bb = p4sq.tile([128, SL], BF16, tag="r2b", bufs=1)
            nc.vector.tensor_copy(mu2b, mu2)
            nc.vector.tensor_copy(r2Bb, r2B)
            # pre-normalize src2b in place: LN2 folded into the operand
            for blk in range(8):
                eng = nc.vector if blk % 2 == 0 else nc.gpsimd
                sb = src2b_sb[:, blk, :]
                eng.tensor_sub(sb, sb, mu2b)
                eng.tensor_mul(sb, sb, r2Bb)
        p4in.release()
        p4w.release()

        # =============== P6: FFN (LN2 folded in) ===============
        with (
            tc.tile_pool(name="p6r", bufs=1) as p6r,
            tc.tile_pool(name="p6w", bufs=2) as p6w,
            tc.tile_pool(name="p6ps", bufs=4, space="PSUM") as p6ps,
            tc.tile_pool(name="p6ps2", bufs=2, space="PSUM") as p6ps2,
        ):
            relu_sb = p6r.tile([128, 32, SL], BF16)
            for fb in range(32):
                ps = p6ps.tile([128, SL], F32, tag="f1")
                for c in range(8):
                    nc.tensor.matmul(ps, w1_sb[:, c, ts(fb, 128)],
                                     src2b_sb[:, c, :],
                                     start=(c == 0), stop=(c == 7))
                if use_c1:
                    nc.scalar.activation(relu_sb[:, fb, :], ps, AF.Relu,
                                         bias=c1_sb[:, fb:fb + 1])
                else:
                    nc.scalar.activation(relu_sb[:, fb, :], ps, AF.Relu)
            outT_sb = p6r.tile([128, 8, SL], F32)
            for dblk in range(8):
                w2p = p6w.tile([128, 32, 128], BF16, tag="w2p")
                nc.sync.dma_start(
                    w2p,
                    w2T.ap()[:, ts(dblk, 128)].rearrange("(c p) d -> p c d", p=128))
                ps = p6ps2.tile([128, SL], F32, tag="f2")
                for c in range(32):
                    nc.tensor.matmul(ps, w2p[:, c, :],
                                     relu_sb[:, c, :],
                                     start=(c == 0), stop=(c == 31))
                if use_b2:
                    nc.vector.scalar_tensor_tensor(
                        outT_sb[:, dblk, :], ps, b2_sb[:, dblk:dblk + 1],
                        src2T_sb[:, dblk, :], op0=ALU.add, op1=ALU.add)
                else:
                    nc.vector.tensor_add(outT_sb[:, dblk, :], ps,
                                         src2T_sb[:, dblk, :])
                nc.sync.dma_start(
                    outT.ap().rearrange("(g p) t -> p g t", p=128)[:, dblk, :],
                    outT_sb[:, dblk, :])
        pw1.release()
        ln2.release()
        p46.release()
        dram.release()
        warm_dram.release()
        consts.release()

    nc.compile()
    return nc


_CACHE = {}


def _get_nc(flags):
    if flags not in _CACHE:
        _CACHE[flags] = build_nc(*flags)
    return _CACHE[flags]


def _bf16(a):
    return np.ascontiguousarray(a.astype(ml_dtypes.bfloat16))


def prep_in_maps(src, ln1_g, ln1_b, Wqkv, bqkv, Wout, bout, ln2_g, ln2_b,
                 W1, b1, W2, b2):
    src = np.asarray(src, np.float32)
    ln1_g = np.asarray(ln1_g, np.float32); ln1_b = np.asarray(ln1_b, np.float32)
    Wqkv = np.asarray(Wqkv, np.float32); bqkv = np.asarray(bqkv, np.float32)
    Wout = np.asarray(Wout, np.float32); bout = np.asarray(bout, np.float32)
    ln2_g = np.asarray(ln2_g, np.float32); ln2_b = np.asarray(ln2_b, np.float32)
    W1 = np.asarray(W1, np.float32); b1 = np.asarray(b1, np.float32)
    W2 = np.asarray(W2, np.float32); b2 = np.asarray(b2, np.float32)

    WqkvT_g = ln1_g[:, None] * Wqkv.T          # [D, 3D]
    const_qkv = ln1_b @ Wqkv.T + bqkv          # [3D]
    w1T_g = ln2_g[:, None] * W1.T              # [D, FF]
    c1_full = ln2_b @ W1.T + b1                # [FF]
    w2T = _bf16(W2.T)                          # [FF, D]
    # chunk c of the received attention output holds heads
    # (h_lo, h_hi) = (4+rr, 8+rr) for pair 0, (12+rr, rr) for pair 1
    # (rr = c % 4); permute Wout^T rows to match that order.
    perm = np.empty(D, np.int64)
    selb_np = np.zeros((8, 8 * 128), np.float32)
    for c in range(8):
        pr, rr = c // 4, c % 4
        h_lo = 4 + rr if pr == 0 else 12 + rr
        h_hi = 8 + rr if pr == 0 else rr
        perm[c * 128:c * 128 + 64] = np.arange(64 * h_lo, 64 * h_lo + 64)
        perm[c * 128 + 64:c * 128 + 128] = np.arange(64 * h_hi,
                                                     64 * h_hi + 64)
        selb_np[2 * rr + 0, c * 128:c * 128 + 64] = 1.0
        selb_np[2 * rr + 1, c * 128 + 64:c * 128 + 128] = 1.0
    woutT = _bf16(Wout.T[perm, :])             # [D, D] (rows permuted)
    selb_np = _bf16(selb_np)
    cs1_row = _bf16(w1T_g.sum(axis=0).reshape(1, FF))
    b2col = np.ascontiguousarray(b2.reshape(8, 128).T)        # [128, 8]
    c1col = np.ascontiguousarray(c1_full.reshape(32, 128).T)  # [128, 32]

    tri_np = (np.arange(128)[:, None] <= np.arange(128)[None, :])
    tri_np = _bf16(tri_np.astype(np.float32))

    use_c1 = bool(np.any(c1_full))
    use_b2 = bool(np.any(b2))

    in_maps = []
    use_cqk = use_cv = False
    for c in range(NCORES):
        b, r = c // GROUP, c % GROUP
        # slot order: [full, full, full, banded-steep]; slot 3 gets the
        # steep-alibi head r (kb-banded in the kernel)
        heads = [4 + r, 8 + r, 12 + r, r]
        qcols = np.concatenate([np.arange(DH * h, DH * h + DH) for h in heads])
        kcols = qcols + D
        vcols = qcols + 2 * D
        qkcols = np.concatenate([qcols, kcols])
        srcTb_np = _bf16(src[b].T)
        srcTs_np = np.ascontiguousarray(src[b].T[:, SL * r:SL * r + SL]
                                        + bout[:, None])
        wqkT_np = _bf16(WqkvT_g[:, qkcols])
        wvT_np = _bf16(WqkvT_g[:, vcols])
        csqk_np = _bf16(WqkvT_g[:, qkcols].sum(axis=0).reshape(1, -1))
        # negated: the v post-op computes (csv * mu) + psum with mu (not -mu)
        csv_np = np.ascontiguousarray(
            -WqkvT_g[:, vcols].sum(axis=0).reshape(1, -1))
        cqk_np = np.ascontiguousarray(const_qkv[qkcols].reshape(4, 128).T)
        cv_np = np.ascontiguousarray(const_qkv[vcols].reshape(1, HPC * DH))
        if np.any(cqk_np):
            use_cqk = True
        if np.any(cv_np):
            use_cv = True
        ali = np.zeros((128, HPC * 16), np.float32)
        j = np.arange(128)
        for hl, hh in enumerate(heads):
            slope = 2.0 ** (-float(hh))
            for kb in range(16):
                ali[:, hl * 16 + kb] = -slope * (kb * 128 + j)
        # receive-side gather from a2a_outs[pair] flat [(s hh p), 512]:
        # row = s*130 + hh*65 + p  (s = 4g + rr)
        g = c // GROUP
        gidx_np = np.empty((128, 8), np.int32)
        gidxd_np = np.empty((8, 2), np.int32)
        for cp in range(8):
            rr = cp % 4
            p = np.arange(128)
            gidx_np[:, cp] = ((4 * g + rr) * 130 + (p // 64) * 65 + (p % 64))
        for pr in range(2):
            j = np.arange(8)
            gidxd_np[:, pr] = (4 * g + j // 2) * 130 + (j % 2) * 65 + 64
        in_maps.append(dict(
            srcTb=srcTb_np, srcTs=srcTs_np, wqkT=wqkT_np, wvT=wvT_np,
            woutT=woutT, w1T=_bf16(w1T_g), w2T=w2T,
            alibi=np.ascontiguousarray(ali), tri=tri_np,
            csqk=csqk_np, csv=csv_np, cs1=cs1_row,
            cqk=cqk_np, cv=cv_np, c1=c1col, b2c=b2col,
            gidx=np.ascontiguousarray(gidx_np),
            gidxd=np.ascontiguousarray(gidxd_np),
            selb=selb_np))

    return in_maps, (use_cqk, use_cv, use_c1, use_b2)


def kernel(**inputs):
    _want_trace = inputs.pop("_want_trace", False)
    in_maps, flags = prep_in_maps(**inputs)
    nc = _get_nc(flags)
    # filter to the inputs the compiled program actually kept
    expected = set()
    for alloc in nc.m.functions[0].allocations:
        if isinstance(alloc, mybir.MemoryLocationSet) and \
           alloc.kind == "ExternalInput":
            expected.add(alloc.memorylocations[0].name)
    in_maps = [{k: v for k, v in m.items() if k in expected} for m in in_maps]

    res = run_bass_kernel_spmd(nc, in_maps, core_ids=list(range(NCORES)),
                               trace=_want_trace)
    out = np.empty((B, S, D), np.float32)
    for c in range(NCORES):
        b, r = c // GROUP, c % GROUP
        out[b, SL * r:SL * r + SL, :] = res.results[c]["outT"].T
    if _want_trace:
        return out, res
    return out

